# revision 1
# baseline (speedup 1.0000x reference)
"""TRN2 Bass kernel for nn_CropLayer (crop_and_resize, bilinear, 28x28).

Contract: kernel(images, boxes) takes the FULL inputs
  images [8, 512, 512, 32] f32, boxes [8, 100, 4] f32
and returns the FULL output [800, 28, 28, 32] f32, running the compute on
8 NeuronCores (data-parallel over the batch axis, one image per core).

Per-core device program (SPMD; all cores share one compiled program):
  inputs : img  [IMG_ELEMS] f32      one image (flat) + padding
           idxT [128, L//16] i16     wrapped band-local gather indices (top row)
           idxB [128, L//16] i16     (bottom row)
           wts  [128, (L//128)*6] f32  6 blend weights per sample point
  output : out  [L, 32] f32          band-sorted point order; host unsorts

How it works
------------
crop_and_resize needs, per output point, a bilinear blend of a 2x2 pixel
neighbourhood at a data-dependent position.  The kernel fetches, per point,
two 512 B image windows (4 horizontally consecutive pixels x 32 channels,
256 B aligned) - one on image row y0, one on row y0+1 - with the bulk
`dma_gather` SWDGE instruction (thousands of descriptors per instruction,
split over two SWDGE queues).  The gather indices are int16, so the image
is covered by 5 overlapping bands of 128 rows starting at rows
{0,127,254,381,508}: a row pair (y0, y0+1) always lies in one band, and a
band-local window index fits 15 bits.  The host sorts the sample points by
band (each band's list padded to a common size so all 8 cores can share one
program), computes the 6 folded blend weights per point
  (1-ly)*a_s and ly*a_s,  a_s the x-lerp weight of window slice s in {0,1,2}
with the pair-parity and validity mask folded in, and un-sorts the device
output on the way back.  On device each 6272-point chunk is: 2+ dma_gather,
11 DVE tensor_tensor ops, one dense 0.8 MB store.

Everything data-dependent about the *schedule* (per-band point counts) is
JIT-computed from the actual inputs at call time; the device program is
rebuilt (and cached) per schedule.
"""

import sys

if '/opt/trn_rl_repo' not in sys.path:
    sys.path.insert(0, '/opt/trn_rl_repo')

import numpy as np

import concourse.bacc as bacc
import concourse.mybir as mybir
import concourse.tile as tile
import concourse.tile_sem_assignment as tsa
from concourse.ap import AP

F32 = mybir.dt.float32
I16 = mybir.dt.int16
OP = mybir.AluOpType

P = 128
H = 512
W = 512
C = 32
CROP = 28
NB = 100
PTS = CROP * CROP
NPT = NB * PTS
CHUNK = 6272              # points per chunk (= 128 * 49)
MCOL = CHUNK // P
BAND_STRIDE = 127
NBANDS = 5
# band-3 gather window AP nominally spans offset + 32768*128 elements;
# CoreSim and the DMA-AP bounds validation see the full nominal extent
IMG_ELEMS = 3 * BAND_STRIDE * W * C + 32768 * 4 * C

# ---------------------------------------------------------------------------
# Tile round-robins Pool-engine DMA insts over all 8 DMASW sem lanes with no
# regard for the SWDGE queue they run on, but each lane may only be updated
# from one queue (ucode constraint, enforced by CoreSim).  Patch the lane
# assignment so queue q only ever gets lanes with lane % 2 == q.
_orig_assign_tick = tsa.TileClockTick._assign_tick
_IDXQ = {}


def _queue_aware_assign_tick(self, inst):
    qn = getattr(inst, "queue_num", None)
    if isinstance(inst, mybir.InstDMAGatherAnt) and qn is not None:
        ctr = _IDXQ.setdefault(id(self), {})
        c = ctr.get(qn, 0)
        ctr[qn] = c + 1
        self.next_sw_dma_idx = (qn + 2 * c) % self.swdge_sem_count
    return _orig_assign_tick(self, inst)


if tsa.TileClockTick._assign_tick is not _queue_aware_assign_tick:
    tsa.TileClockTick._assign_tick = _queue_aware_assign_tick


# ---------------------------------------------------------------------------
def _host_point_data(boxes_core):
    """Per-core box math in f32, mirroring the reference op-for-op."""
    b = boxes_core.astype(np.float32)
    y1, x1, y2, x2 = b[:, 0], b[:, 1], b[:, 2], b[:, 3]
    g = np.arange(CROP, dtype=np.float32)
    hsc = (y2 - y1) * np.float32(H - 1) / np.float32(CROP - 1)
    wsc = (x2 - x1) * np.float32(W - 1) / np.float32(CROP - 1)
    in_y = y1[:, None] * np.float32(H - 1) + g[None, :] * hsc[:, None]
    in_x = x1[:, None] * np.float32(W - 1) + g[None, :] * wsc[:, None]

    vy = (in_y >= 0) & (in_y <= H - 1)
    vx = (in_x >= 0) & (in_x <= W - 1)
    y0f = np.floor(in_y)
    x0f = np.floor(in_x)
    ly = in_y - y0f
    lx = in_x - x0f
    y0 = np.clip(y0f.astype(np.int32), 0, H - 1)
    x0 = np.clip(x0f.astype(np.int32), 0, W - 1)

    band = np.minimum(y0 // BAND_STRIDE, NBANDS - 1)
    ylocal = y0 - band * BAND_STRIDE
    ybot_local = np.minimum(ylocal + 1, (H - 1) - band * BAND_STRIDE)

    xaddr = np.minimum(x0, W - 2)
    win = xaddr >> 1
    par = (xaddr & 1).astype(np.float32)

    idx_top = (ylocal[:, :, None] * (W // 2) + win[:, None, :]).astype(np.int16)
    idx_bot = (ybot_local[:, :, None] * (W // 2) + win[:, None, :]).astype(np.int16)

    a0 = (1 - par) * (1 - lx)
    a1 = (1 - par) * lx + par * (1 - lx)
    a2 = par * lx
    mask = (vy[:, :, None] & vx[:, None, :]).astype(np.float32)
    omly = (1 - ly)[:, :, None, None]
    lyb = ly[:, :, None, None]
    ax = np.stack([a0, a1, a2], axis=-1)[:, None, :, :]
    w6 = np.concatenate([omly * ax, lyb * ax], axis=-1)
    w6 = w6 * mask[:, :, :, None]
    return band, idx_top, idx_bot, w6.astype(np.float32)


def _make_schedule(images, boxes):
    B = images.shape[0]
    per_core = []
    band_counts = np.zeros((B, NBANDS), np.int64)
    for c in range(B):
        band, it, ib, w6 = _host_point_data(boxes[c])
        per_core.append((band, it, ib, w6))
        for k in range(NBANDS):
            band_counts[c, k] = int((band == k).sum()) * CROP

    pk = band_counts.max(axis=0)
    pk = ((pk + P - 1) // P) * P
    total = int(pk.sum())
    L = ((total + CHUNK - 1) // CHUNK) * CHUNK
    last = int(np.nonzero(pk)[0][-1]) if pk.sum() else 0
    pk[last] += L - total

    segments = [[] for _ in range(L // CHUNK)]
    off = 0
    for k in range(NBANDS):
        remaining = int(pk[k])
        while remaining > 0:
            ch = off // CHUNK
            room = CHUNK - (off % CHUNK)
            take = min(room, remaining)
            segments[ch].append((k, off % CHUNK, take))
            off += take
            remaining -= take

    in_maps = []
    unsort_rows = []
    for c in range(B):
        band, it, ib, w6 = per_core[c]
        flatT = np.zeros(L, np.int16)
        flatB = np.zeros(L, np.int16)
        flatW = np.zeros((L, 6), np.float32)
        pos_of = np.empty(NPT, np.int64)
        off = 0
        for k in range(NBANDS):
            units = np.nonzero((band == k).ravel())[0]
            npts = len(units) * CROP
            if npts:
                n_id = units // CROP
                i_id = units % CROP
                pt_ids = (n_id[:, None] * PTS + i_id[:, None] * CROP
                          + np.arange(CROP)[None, :]).ravel()
                sl = slice(off, off + npts)
                flatT[sl] = it[n_id, i_id].ravel()
                flatB[sl] = ib[n_id, i_id].ravel()
                flatW[sl] = w6[n_id, i_id].reshape(-1, 6)
                pos_of[pt_ids] = np.arange(off, off + npts)
            off += int(pk[k])
        wrapT = flatT.reshape(L // 16, 16).T
        wrapB = flatB.reshape(L // 16, 16).T
        idxT = np.tile(wrapT, (8, 1)).copy()
        idxB = np.tile(wrapB, (8, 1)).copy()
        wts = np.ascontiguousarray(
            flatW.reshape(L // P, P, 6).transpose(1, 0, 2).reshape(P, -1))
        img = np.zeros(IMG_ELEMS, np.float32)
        img[:H * W * C] = np.ascontiguousarray(images[c]).ravel()
        in_maps.append({"img": img, "idxT": idxT, "idxB": idxB, "wts": wts})

        q = pos_of
        ch = q // CHUNK
        ql = q % CHUNK
        unsort_rows.append(ch * CHUNK + (ql % P) * MCOL + ql // P)

    return in_maps, segments, unsort_rows, L


def _build_nc(segments, L, num_devices=8):
    nc = bacc.Bacc("TRN2", target_bir_lowering=False, debug=False,
                   num_devices=num_devices, num_swdge_queues=2)
    img = nc.dram_tensor("img", [IMG_ELEMS], F32, kind="ExternalInput")
    idxT_d = nc.dram_tensor("idxT", [P, L // 16], I16, kind="ExternalInput")
    idxB_d = nc.dram_tensor("idxB", [P, L // 16], I16, kind="ExternalInput")
    wts_d = nc.dram_tensor("wts", [P, (L // P) * 6], F32, kind="ExternalInput")
    out_d = nc.dram_tensor("out", [L, C], F32, kind="ExternalOutput")

    nchunks = L // CHUNK
    outv = out_d.ap().rearrange("(c p q) e -> c p (q e)", c=nchunks, p=P)

    def band_in_ap(k):
        n_avail = min(32768, ((H + 1) - BAND_STRIDE * k) * (W // 2))
        return AP(img, BAND_STRIDE * k * W * C, [[2 * C, n_avail], [1, 4 * C]])

    with tile.TileContext(nc) as tc:
        with tc.tile_pool(name="persist", bufs=1) as pp:
            idxT = pp.tile([P, L // 16], I16)
            idxB = pp.tile([P, L // 16], I16)
            wts = pp.tile([P, (L // P) * 6], F32)
            nc.sync.dma_start(idxT[:], idxT_d.ap())
            nc.sync.dma_start(idxB[:], idxB_d.ap())
            nc.sync.dma_start(wts[:], wts_d.ap())
            wtsv = wts[:].rearrange("p (m s) -> p m s", s=6)

            with tc.tile_pool(name="work", bufs=2) as wp:
                for ci in range(nchunks):
                    T = wp.tile([P, CHUNK], F32, tag="T")
                    B = wp.tile([P, CHUNK], F32, tag="B")
                    T3 = T[:].rearrange("p (m e) -> p m e", e=4 * C)
                    B3 = B[:].rearrange("p (m e) -> p m e", e=4 * C)
                    for (k, s0, cnt) in segments[ci]:
                        gcol = (ci * CHUNK + s0) // 16
                        m0 = s0 // P
                        mw = cnt // P
                        for qn, (idx_sb, dst) in enumerate(
                                ((idxT, T3), (idxB, B3))):
                            nc.gpsimd.dma_gather(
                                out_ap=dst[:, m0:m0 + mw, :],
                                in_ap=band_in_ap(k),
                                idxs_ap=idx_sb[:, gcol:gcol + cnt // 16],
                                num_idxs=cnt,
                                num_idxs_reg=cnt,
                                elem_size=4 * C,
                                elem_step=2 * C,
                                single_packet=False,
                                queue_num=qn,
                            )

                    res = wp.tile([P, MCOL * C], F32, tag="res")
                    tmp = wp.tile([P, MCOL * C], F32, tag="tmp")
                    r3 = res[:].rearrange("p (m e) -> p m e", e=C)
                    t3 = tmp[:].rearrange("p (m e) -> p m e", e=C)

                    def wb(s):
                        return (wtsv[:, ci * MCOL:(ci + 1) * MCOL, s:s + 1]
                                .to_broadcast([P, MCOL, C]))

                    srcs = [(T3, 0), (T3, 1), (T3, 2), (B3, 3), (B3, 4), (B3, 5)]
                    first = True
                    for (G3, s) in srcs:
                        sl = G3[:, :, (s % 3) * C:(s % 3) * C + C]
                        if first:
                            nc.vector.tensor_tensor(r3, sl, wb(s), op=OP.mult)
                            first = False
                        else:
                            nc.vector.tensor_tensor(t3, sl, wb(s), op=OP.mult)
                            nc.vector.tensor_tensor(r3, r3, t3, op=OP.add)
                    nc.sync.dma_start(outv[ci], res[:])

    nc.compile()
    return nc


_NC_CACHE = {}


def kernel(images, boxes):
    images = np.ascontiguousarray(np.asarray(images, dtype=np.float32))
    boxes = np.ascontiguousarray(np.asarray(boxes, dtype=np.float32))
    B = images.shape[0]

    in_maps, segments, unsort_rows, L = _make_schedule(images, boxes)

    key = (B, L, tuple(tuple(s) for cs in segments for s in cs))
    nc = _NC_CACHE.get(key)
    if nc is None:
        nc = _build_nc(segments, L, num_devices=B)
        _NC_CACHE.clear()
        _NC_CACHE[key] = nc

    from concourse import bass_utils
    res = bass_utils.run_bass_kernel_spmd(nc, in_maps, core_ids=list(range(B)))

    outs = []
    for c in range(B):
        scratch = res.results[c]["out"]
        outs.append(scratch[unsort_rows[c]].reshape(NB, CROP, CROP, C))
    return np.concatenate(outs, axis=0)

